# revision 38
# baseline (speedup 1.0000x reference)
"""Trainium2 Bass kernel for nn_MultiHeadAttention_79508434583676.

Reference semantics (faithful to source bugs):
  proj = x @ Wq.T + bq  for x in {Q, K, V}   (Wq projects all three)
  q,k,v = split_heads(proj)                  [B,H,N,dk]
  scores = q @ k.T / sqrt(dk)                [B,H,N,N]
  probs = softmax(scores, axis=1)            (softmax over the HEADS axis)
  A = probs @ v -> combine heads -> A @ Wo.T + bo

Sharding: 8 cores = 4 batches x 2 query-halves. Softmax over heads is local
to each (n,m) score position, so with all heads on one core there is no
cross-core coupling -> no collectives. K/V work for a batch is duplicated
across its 2 cores.

Per-core pipeline (NQ=1024 query rows, NK=2048 key rows, D=512, H=8, dk=64):
  phase 1: PE-transpose Q,K,V tiles (bf16); bf16 projections using host-
           pre-transposed Wq.T; q,k projections kept transposed [e, n];
           v projection kept natural [m, e]. The k-path weights are host
           pre-scaled by 1/sqrt(dk) so exp runs with scale=1.
  phase 2: 2-deep software pipeline over (m-tile 128, n-chunk 512) steps:
           step c runs row-packed score matmuls + ACT exp for step c; the
           cross-head sum for step c-1 as a bf16 add tree split between
           GpSimd (level 1a) and DVE (1b, 2, 3) + reciprocal_approx_fast
           (DVE) + cast (GpSimd) + the broadcast normalize mul split
           6/8 heads DVE, 2/8 GpSimd; and the col-packed A^T accumulation
           matmuls for step c-2 emitted AFTER the scores so the PE queue
           never head-of-line blocks the exp relay. The PE does no
           reduction work -- engines are balanced at ~5us/step each.
           Output projection consumes A^T PSUM tiles directly; + bo; DMA.
"""

import sys

sys.path.insert(0, "/opt/trn_rl_repo")

import math
from contextlib import ExitStack

import numpy as np

import concourse.bass as bass
from concourse.bacc import Bacc
import concourse.mybir as mybir
import concourse.tile as tile
from concourse.masks import make_identity

F32 = mybir.dt.float32
F32R = mybir.dt.float32r
BF16 = mybir.dt.bfloat16
ADD = mybir.AluOpType.add
MULT = mybir.AluOpType.mult

B, N, D, H = 4, 2048, 512, 8
DK = D // H           # 64
NQ = N // 2           # 1024 query rows per core
NK = N                # 2048 key rows per core
NCH = 512             # n-chunk (score matmul free dim)
N_CHUNKS = NQ // NCH  # 2
MT = NK // 128        # 16 m-tiles
ET = D // 128         # 4 e-tiles (= head pairs)
SCALE = 1.0 / math.sqrt(DK)


def r32(ap):
    return ap.bitcast(F32R)


def build_nc(repeat: int | None = None) -> bass.Bass:
    nc = Bacc()

    Qd = nc.dram_tensor("q_in", [NQ, D], F32, kind="ExternalInput")
    Kd = nc.dram_tensor("k_in", [NK, D], F32, kind="ExternalInput")
    Vd = nc.dram_tensor("v_in", [NK, D], F32, kind="ExternalInput")
    WqTd = nc.dram_tensor("wqt", [D, D], F32, kind="ExternalInput")  # Wq.T [d, e]
    WqTsd = nc.dram_tensor("wqts", [D, D], F32, kind="ExternalInput")  # Wq.T/sqrt(dk)
    WoTd = nc.dram_tensor("wot", [D, D], F32, kind="ExternalInput")  # Wo.T [e, eo]
    bqd = nc.dram_tensor("bq", [1, D], F32, kind="ExternalInput")
    bqsd = nc.dram_tensor("bqs", [1, D], F32, kind="ExternalInput")  # bq/sqrt(dk)
    bod = nc.dram_tensor("bo", [1, D], F32, kind="ExternalInput")
    OUT = nc.dram_tensor("out", [NQ, D], F32, kind="ExternalOutput")

    with ExitStack() as ctx:
        tc = ctx.enter_context(tile.TileContext(nc))
        _emit(ctx, tc, Qd, Kd, Vd, WqTd, WqTsd, WoTd, bqd, bqsd, bod, OUT,
              repeat=repeat)

    nc.finalize()
    return nc


def _emit(ctx, tc, Qd, Kd, Vd, WqTd, WqTsd, WoTd, bqd, bqsd, bod, OUT,
          repeat=None):
    nc = tc.nc

    # ---------------------------------------------------------- constants
    const_pool = ctx.enter_context(tc.tile_pool(name="const", bufs=1))

    ident = const_pool.tile([128, 128], F32, name="ident")
    make_identity(nc, ident)
    ident_bf = const_pool.tile([128, 128], BF16, name="ident_bf")
    make_identity(nc, ident_bf)

    # bq with e on partitions: element (p, t) = bq[t*128 + p]
    bq_cols = const_pool.tile([128, ET], F32, name="bq_cols")
    nc.sync.dma_start(bq_cols[:, :], bqd[0, :].rearrange("(t p) -> p t", p=128))
    bqs_cols = const_pool.tile([128, ET], F32, name="bqs_cols")
    nc.sync.dma_start(bqs_cols[:, :], bqsd[0, :].rearrange("(t p) -> p t", p=128))

    bq_bcast = const_pool.tile([128, D], F32, name="bq_bcast")
    bo_bcast = const_pool.tile([128, D], F32, name="bo_bcast")

    wqt_bf = []   # Wq.T bf16 tiles, d on partitions (q/v path)
    wqts_bf = []  # Wq.T/sqrt(dk) bf16 tiles (k path)
    wot_bf = []   # Wo.T bf16 tiles, e on partitions
    for t in range(ET):
        wqt_bf.append(
            const_pool.tile([128, D], BF16, name=f"wqtb{t}", tag=f"wqtb{t}")
        )
        wqts_bf.append(
            const_pool.tile([128, D], BF16, name=f"wqtsb{t}", tag=f"wqtsb{t}")
        )
        wot_bf.append(
            const_pool.tile([128, D], BF16, name=f"wotb{t}", tag=f"wotb{t}")
        )

    with tc.tile_pool(name="setup_stage", bufs=2) as sstage:
        for bias_d, dst in ((bqd, bq_bcast), (bod, bo_bcast)):
            nc.sync.dma_start(dst[:, :], bias_d[0, :].partition_broadcast(128))
        for src_d, dsts in ((WqTd, wqt_bf), (WqTsd, wqts_bf), (WoTd, wot_bf)):
            for t in range(ET):
                wstage = sstage.tile([128, D], F32, name="wstage", tag="wstage")
                nc.sync.dma_start(wstage[:, :], src_d[t * 128 : (t + 1) * 128, :])
                nc.vector.tensor_copy(dsts[t][:, :], wstage[:, :])

    # --------------------------------------------------- persistent SBUF
    qp_pool = ctx.enter_context(tc.tile_pool(name="qp", bufs=ET))
    kp_pool = ctx.enter_context(tc.tile_pool(name="kp", bufs=ET))
    vp_pool = ctx.enter_context(tc.tile_pool(name="vp", bufs=MT))
    qpT = [qp_pool.tile([128, NQ], BF16, name=f"qpT{t}", tag="qpT") for t in range(ET)]
    kpT = [kp_pool.tile([128, NK], BF16, name=f"kpT{t}", tag="kpT") for t in range(ET)]
    vp = [vp_pool.tile([128, D], BF16, name=f"vp{m}", tag="vp") for m in range(MT)]

    # ----------------------------------------------------------- phase 1
    def load_transpose(stage_pool, ps_pool, Xd, xT_all, n_rows):
        """DMA [n_rows, D] fp32 from DRAM, PE-transpose (bf16) into a single
        [128, ET*n_rows] tensor (d-tile-major along free); one scatter-copy
        evacuation per 128-row block."""
        xT3 = xT_all[:, :].rearrange("p (t n) -> p t n", t=ET)
        for ntile in range(n_rows // 128):
            st = stage_pool.tile([128, D], F32, name="x_stage", tag="stage")
            dma_eng = nc.sync if ntile % 2 == 0 else nc.scalar
            dma_eng.dma_start(st[:, :], Xd[ntile * 128 : (ntile + 1) * 128, :])
            st_bf = stage_pool.tile([128, D], BF16, name="x_stage_bf", tag="stage_bf")
            nc.scalar.copy(st_bf[:, :], st[:, :])
            ps = ps_pool.tile([128, D], BF16, name="ps_tr", tag="ps_s")
            for dt_ in range(ET):
                nc.tensor.transpose(
                    ps[:, dt_ * 128 : (dt_ + 1) * 128],
                    st_bf[:, dt_ * 128 : (dt_ + 1) * 128],
                    ident_bf[:, :],
                )
            nc.vector.tensor_copy(
                xT3[:, :, ntile * 128 : (ntile + 1) * 128],
                ps[:, :].rearrange("p (t n) -> p t n", t=ET),
            )

    def project_T(ps_pool, xT_all, xpT, n_rows, w_bf, b_cols, nchs=None,
                  ps_tag="psA", ets=None):
        """xpT[et][e, n] = sum_d W[d, e] xT[d, n] + b[e]  (bf16)."""
        if nchs is None:
            nchs = range(n_rows // NCH)
        if ets is None:
            ets = range(ET)
        for et in ets:
            for nch in nchs:
                ps = ps_pool.tile([128, NCH], F32, name="ps_proj", tag=ps_tag)
                for dt_ in range(ET):
                    base = dt_ * n_rows + nch * NCH
                    nc.tensor.matmul(
                        ps[:, :],
                        w_bf[dt_][:, et * 128 : (et + 1) * 128],
                        xT_all[:, base : base + NCH],
                        start=(dt_ == 0),
                        stop=(dt_ == ET - 1),
                    )
                nc.vector.tensor_scalar_add(
                    xpT[et][:, nch * NCH : (nch + 1) * NCH],
                    ps[:, :],
                    b_cols[:, et : et + 1],
                )

    stage_pool = ctx.enter_context(tc.tile_pool(name="stage", bufs=4))
    xtq_pool = ctx.enter_context(tc.tile_pool(name="xtq", bufs=1))
    xtk_pool = ctx.enter_context(tc.tile_pool(name="xtk", bufs=1))
    e_pool = ctx.enter_context(tc.tile_pool(name="ework", bufs=5))
    p_pool = ctx.enter_context(tc.tile_pool(name="pwork", bufs=5))
    r_pool = ctx.enter_context(tc.tile_pool(name="rwork", bufs=2))
    a_pool = ctx.enter_context(tc.tile_pool(name="abuf", bufs=ET))
    o_pool = ctx.enter_context(tc.tile_pool(name="ostage", bufs=2))
    # PSUM: 8 banks total. ps_s pool: 2 slots x [128,1024]f32 (2 banks each);
    # phase-1 transposes + out-proj share the slots via the same tag.
    # ps_a pool: 4 slots x 1 bank; phase-1 projections share via tag.
    ps_s_pool = ctx.enter_context(tc.tile_pool(name="ps_s", bufs=2, space="PSUM"))
    ps_a_pool = ctx.enter_context(tc.tile_pool(name="ps_a", bufs=ET, space="PSUM"))

    ps_t_pool = ps_s_pool
    ps_p_pool = ps_a_pool

    def body():
        # K first: scores need the full kpT before step 0
        kT = xtk_pool.tile([128, ET * NK], BF16, name="kT", tag="kT")
        load_transpose(stage_pool, ps_t_pool, Kd, kT, NK)
        project_T(ps_p_pool, kT, kpT, NK, wqts_bf, bqs_cols)

        # Q: transpose all tiles, but project only the nch=0 half now;
        # the nch=1 half is projected mid-loop before it is needed
        qT = xtq_pool.tile([128, ET * NQ], BF16, name="qT", tag="qT")
        load_transpose(stage_pool, ps_t_pool, Qd, qT, NQ)
        project_T(ps_p_pool, qT, qpT, NQ, wqt_bf, bq_cols, nchs=(0,))

        def emit_v_tile(m):
            """Stream one V m-tile: load, cast (DVE), PE-transpose,
            project into vp[m]. Runs inside the phase-2 loop; vp[m] is
            first consumed by A at step m + A_LAG."""
            st = stage_pool.tile([128, D], F32, name="x_stage", tag="stage")
            dma_eng = nc.sync if m % 2 == 0 else nc.scalar
            dma_eng.dma_start(st[:, :], Vd[m * 128 : (m + 1) * 128, :])
            st_bf = stage_pool.tile([128, D], BF16, name="x_stage_bf", tag="stage_bf")
            nc.scalar.copy(st_bf[:, :], st[:, :])
            tr = stage_pool.tile([128, D], BF16, name="v_tr", tag="v_tr")
            ps_tr = ps_t_pool.tile([128, D], BF16, name="ps_tr", tag="ps_s")
            for dt_ in range(ET):
                nc.tensor.transpose(
                    ps_tr[:, dt_ * 128 : (dt_ + 1) * 128],
                    st_bf[:, dt_ * 128 : (dt_ + 1) * 128],
                    ident_bf[:, :],
                )
            nc.scalar.copy(tr[:, :], ps_tr[:, :])
            ps = ps_t_pool.tile([128, D], F32, name="ps_vp", tag="ps_s")
            for dt_ in range(ET):
                nc.tensor.matmul(
                    ps[:, :],
                    tr[:, dt_ * 128 : (dt_ + 1) * 128],
                    wqt_bf[dt_][:, :],
                    start=(dt_ == 0),
                    stop=(dt_ == ET - 1),
                )
            nc.vector.tensor_tensor(vp[m][:, :], ps[:, :], bq_bcast[:, :], ADD)

        # ------------------------------------------------------- phase 2
        def emit_A_half(psA, mt, Phalves, j):
            """A^T accumulation for head pairs (2j, 2j+1); col-packed.
            Emitted interleaved into the score relay's PE idle gaps."""
            for pair in (2 * j, 2 * j + 1):
                Ph = Phalves[pair // 2]
                hbase = (pair % 2) * 2
                for half in range(2):
                    h = 2 * pair + half
                    nc.tensor.matmul(
                        psA[pair][64 * half : 64 * (half + 1), :],
                        vp[mt][:, h * DK : (h + 1) * DK],
                        Ph[:, (hbase + half) * NCH : (hbase + half + 1) * NCH],
                        start=(mt == 0),
                        stop=(mt == MT - 1),
                        tile_position=(0, 64 * half),
                        # the sim's zero-region tracker can't see the
                        # partition offset; the two col-packed halves of
                        # one bank are distinct accumulation groups
                        skip_group_check=True,
                    )

        def emit_score_pair(E, nch, mt, pair):
            """Row-packed score matmuls + exp for one head pair."""
            nsl = slice(nch * NCH, (nch + 1) * NCH)
            msl = slice(mt * 128, (mt + 1) * 128)
            ps_s = ps_s_pool.tile([128, 2 * NCH], F32, name="ps_s", tag="ps_s")
            for half in range(2):
                hsl = slice(64 * half, 64 * (half + 1))
                nc.tensor.matmul(
                    ps_s[:, half * NCH : (half + 1) * NCH],
                    kpT[pair][hsl, msl],
                    qpT[pair][hsl, nsl],
                    tile_position=(64 * half, 0),
                )
            nc.scalar.activation(
                E[:, pair * 2 * NCH : (pair + 1) * 2 * NCH],
                ps_s[:, :],
                mybir.ActivationFunctionType.Exp,
            )

        def emit_norm(E):
            """Cross-head sum: two parallel SBUF->SBUF DMA copies pull the
            E halves out (so the E buffer frees right away and the exp
            relay never stalls on it), one DMA accumulate folds them, then
            DVE + GpSimd merge adds, reciprocal + cast + the broadcast
            normalize mul (DVE). No PE work. Returns P halves."""
            T1 = r_pool.tile([128, 4 * NCH], BF16, name="Tsum1", tag="Tsum1")
            nc.gpsimd.dma_start(T1[:, :], E[:, 0 : 4 * NCH])
            # T1 = (b0+b4 | b1+b5 | b2+b6 | b3+b7); E's last reader is
            # this accum's source -- E frees after just two chain links
            nc.gpsimd.dma_start(
                T1[:, :], E[:, 4 * NCH : 8 * NCH], accum_op=ADD
            )
            # fold the four NCH blocks: u = B0+B1, u2 = B2+B3 (DVE, bf16)
            u = r_pool.tile([128, NCH], BF16, name="u_sum", tag="u_sum")
            nc.vector.tensor_tensor(
                u[:, :], T1[:, 0:NCH], T1[:, NCH : 2 * NCH], ADD
            )
            u2 = r_pool.tile([128, NCH], BF16, name="u2_sum", tag="u2_sum")
            nc.vector.tensor_tensor(
                u2[:, :], T1[:, 2 * NCH : 3 * NCH], T1[:, 3 * NCH : 4 * NCH], ADD
            )
            # S = u + u2  (f32, on GpSimd -- SBUF only)
            s_f = r_pool.tile([128, NCH], F32, name="s_f", tag="s_f")
            nc.gpsimd.tensor_tensor(s_f[:, :], u[:, :], u2[:, :], ADD)
            r_f = r_pool.tile([128, NCH], F32, name="r_f", tag="r_f")
            nc.vector.reciprocal_approx_fast(r_f[:, :], s_f[:, :])
            r_bf = r_pool.tile([128, NCH], BF16, name="r_bf", tag="r_bf")
            nc.vector.tensor_copy(r_bf[:, :], r_f[:, :])
            # normalize in two half tiles so downstream A matmuls can
            # start after the first half
            HH = H // 2
            Phalves = []
            for j in range(2):
                Ph = p_pool.tile([128, HH * NCH], BF16, name=f"P{j}", tag=f"P{j}")
                nc.vector.tensor_tensor(
                    Ph[:, :].rearrange("p (h n) -> p h n", h=HH),
                    E[:, j * HH * NCH : (j + 1) * HH * NCH].rearrange(
                        "p (h n) -> p h n", h=HH
                    ),
                    r_bf[:, None, :].broadcast_to([128, HH, NCH]),
                    MULT,
                )
                Phalves.append(Ph)
            return Phalves

        def emit_norm_pe(E):
            """Drain-time norm: head-sum via PE identity-matmul
            accumulation (the PE is idle during the drain; avoids the
            DMA-chain latency), then the usual recip/cast/mul chain."""
            ps_sum = ps_s_pool.tile([128, NCH], F32, name="ps_sum", tag="ps_s")
            for h in range(H):
                nc.tensor.matmul(
                    ps_sum[:, :],
                    ident_bf[:, :],
                    E[:, h * NCH : (h + 1) * NCH],
                    start=(h == 0),
                    stop=(h == H - 1),
                )
            r_f = r_pool.tile([128, NCH], F32, name="r_f", tag="r_f")
            nc.vector.reciprocal_approx_fast(r_f[:, :], ps_sum[:, :])
            r_bf = r_pool.tile([128, NCH], BF16, name="r_bf", tag="r_bf")
            nc.vector.tensor_copy(r_bf[:, :], r_f[:, :])
            HH = H // 2
            Phalves = []
            for j in range(2):
                Ph = p_pool.tile([128, HH * NCH], BF16, name=f"P{j}", tag=f"P{j}")
                nc.vector.tensor_tensor(
                    Ph[:, :].rearrange("p (h n) -> p h n", h=HH),
                    E[:, j * HH * NCH : (j + 1) * HH * NCH].rearrange(
                        "p (h n) -> p h n", h=HH
                    ),
                    r_bf[:, None, :].broadcast_to([128, HH, NCH]),
                    MULT,
                )
                Phalves.append(Ph)
            return Phalves

        def emit_evac(psA):
            # evacuate A^T: psA[pair] partitions = e-rows 128*pair..+127
            a_bf = [
                a_pool.tile([128, NCH], BF16, name=f"a_bf{p}", tag="a_bf")
                for p in range(ET)
            ]
            # GpSimd cannot read PSUM; evacuation stays on DVE
            for p in range(ET):
                nc.vector.tensor_copy(a_bf[p][:, :], psA[p][:, :])
            return a_bf

        def emit_out(nch, a_bf, nt2):
            # output projection: out[n, eo] = sum_e A^T[e, n] WoT[e, eo] + bo
            ps_o = ps_s_pool.tile([128, D], F32, name="ps_o", tag="ps_s")
            for p in range(ET):
                nc.tensor.matmul(
                    ps_o[:, :],
                    a_bf[p][:, nt2 * 128 : (nt2 + 1) * 128],
                    wot_bf[p][:, :],
                    start=(p == 0),
                    stop=(p == ET - 1),
                )
            o_st = o_pool.tile([128, D], F32, name="o_st", tag="o_st")
            nc.vector.tensor_tensor(o_st[:, :], ps_o[:, :], bo_bcast[:, :], ADD)
            nc.sync.dma_start(
                OUT[nch * NCH + nt2 * 128 : nch * NCH + (nt2 + 1) * 128, :],
                o_st[:, :],
            )

        # deep software pipeline over all (n-chunk, m-tile) steps:
        #   step c: scores+exp(c) | norm(c-2) | A(c-4)
        # The DMA-accum head-sum chain has ~7us latency and the full norm
        # chain ~12us; lag 2 for norm and lag 4 for A absorb it. A(c-4) is
        # emitted after the scores so the PE queue never head-of-line
        # blocks the score->exp relay.
        NORM_LAG = 3
        A_LAG = 5
        steps = [(nch, mt) for nch in range(N_CHUNKS) for mt in range(MT)]
        psA_of = {}
        pend_norm = []  # [(nch, mt, E), ...]  emit_norm at lag NORM_LAG
        pend_A = []     # [(nch, mt, P), ...]  emit_A at lag A_LAG

        pend_fin = []  # [(nch, a_bf, nt2), ...] outproj spread 1/step

        def emit_step(ci, nch, mt, arec):
            """One pipeline step: the 4 score pairs + exps of (nch, mt)
            with the A(c-A_LAG) half-batches interleaved into the score
            relay's PE idle gaps, then the streamed V tile / spread-out
            deferred q projection / one spread-out output-projection
            slice, then the norm(c-NORM_LAG) chain."""
            E = e_pool.tile([128, H * NCH], BF16, name="E", tag="E")
            emit_score_pair(E, nch, mt, 0)
            emit_score_pair(E, nch, mt, 1)
            if arec is not None:
                a_nch, a_mt, a_P = arec
                emit_A_half(psA_of[a_nch], a_mt, a_P, 0)
            emit_score_pair(E, nch, mt, 2)
            if arec is not None:
                emit_A_half(psA_of[a_nch], a_mt, a_P, 1)
                if a_mt == MT - 1:
                    a_bf = emit_evac(psA_of[a_nch])
                    pend_fin.extend(
                        (a_nch, a_bf, nt2) for nt2 in range(NCH // 128)
                    )
            emit_score_pair(E, nch, mt, 3)
            if ci is not None and ci < MT:
                emit_v_tile(ci)  # vp[ci] first consumed at step ci+A_LAG
            if ci is not None and 8 <= ci < 8 + ET:
                # project one e-tile of the q second half per step
                # (spread to avoid a PE spike). Must NOT use the psA tag
                # (held by live A accumulators).
                project_T(ps_s_pool, qT, qpT, NQ, wqt_bf, bq_cols, nchs=(1,),
                          ps_tag="ps_s", ets=(ci - 8,))
            if pend_fin:
                emit_out(*pend_fin.pop(0))
            return E

        for ci, (nch, mt) in enumerate(steps):
            if mt == 0:
                psA_of[nch] = [
                    ps_a_pool.tile([128, NCH], F32, name=f"psA{p}", tag="psA")
                    for p in range(ET)
                ]
            arec = pend_A.pop(0) if len(pend_A) > A_LAG - NORM_LAG - 1 else None
            E = emit_step(ci, nch, mt, arec)
            pend_norm.append((nch, mt, E))
            if len(pend_norm) > NORM_LAG:
                n_nch, n_mt, n_E = pend_norm.pop(0)
                pend_A.append((n_nch, n_mt, emit_norm(n_E)))
        # drain: the PE is idle here, so the remaining head-sums run as
        # identity-matmul accumulations (no DMA-chain latency)
        while pend_norm:
            n_nch, n_mt, n_E = pend_norm.pop(0)
            pend_A.append((n_nch, n_mt, emit_norm_pe(n_E)))
        for a_nch, a_mt, a_P in pend_A:
            emit_A_half(psA_of[a_nch], a_mt, a_P, 0)
            emit_A_half(psA_of[a_nch], a_mt, a_P, 1)
            if a_mt == MT - 1:
                a_bf = emit_evac(psA_of[a_nch])
                pend_fin.extend((a_nch, a_bf, nt2) for nt2 in range(NCH // 128))
        for rec in pend_fin:
            emit_out(*rec)

    if repeat:
        # timing variant: loop the whole kernel on-device so execution time
        # dominates the (noisy, ~1ms) per-dispatch tunnel overhead
        with tc.For_i(0, repeat, 1):
            body()
    else:
        body()


# ---------------------------------------------------------------------------
# host wrapper

_CACHED = {}


def _get_nc():
    if "nc" not in _CACHED:
        _CACHED["nc"] = build_nc()
    return _CACHED["nc"]


def make_in_maps(Q, K, V, Wq, bq, Wo, bo):
    Q = np.asarray(Q, dtype=np.float32)
    K = np.asarray(K, dtype=np.float32)
    V = np.asarray(V, dtype=np.float32)
    WqT = np.ascontiguousarray(np.asarray(Wq, np.float32).T)
    WqTs = np.ascontiguousarray(WqT * np.float32(SCALE))
    WoT = np.ascontiguousarray(np.asarray(Wo, np.float32).T)
    bq = np.ascontiguousarray(np.asarray(bq, np.float32)).reshape(1, D)
    bqs = np.ascontiguousarray(bq * np.float32(SCALE))
    bo = np.ascontiguousarray(np.asarray(bo, np.float32)).reshape(1, D)

    in_maps = []
    for c in range(8):
        b, half = divmod(c, 2)
        in_maps.append(
            {
                "q_in": np.ascontiguousarray(Q[b, half * NQ : (half + 1) * NQ]),
                "k_in": np.ascontiguousarray(K[b]),
                "v_in": np.ascontiguousarray(V[b]),
                "wqt": WqT,
                "wqts": WqTs,
                "wot": WoT,
                "bq": bq,
                "bqs": bqs,
                "bo": bo,
            }
        )
    return in_maps


def kernel(Q, K, V, Wq, bq, Wo, bo):
    from concourse import bass_utils

    nc = _get_nc()
    in_maps = make_in_maps(Q, K, V, Wq, bq, Wo, bo)
    res = bass_utils.run_bass_kernel_spmd(nc, in_maps, core_ids=list(range(8)))

    out = np.empty((B, N, D), np.float32)
    for c in range(8):
        b, half = divmod(c, 2)
        out[b, half * NQ : (half + 1) * NQ] = res.results[c]["out"]
    return out



# revision 40
# speedup vs baseline: 1.0242x; 1.0242x over previous
"""Trainium2 Bass kernel for nn_MultiHeadAttention_79508434583676.

Reference semantics (faithful to source bugs):
  proj = x @ Wq.T + bq  for x in {Q, K, V}   (Wq projects all three)
  q,k,v = split_heads(proj)                  [B,H,N,dk]
  scores = q @ k.T / sqrt(dk)                [B,H,N,N]
  probs = softmax(scores, axis=1)            (softmax over the HEADS axis)
  A = probs @ v -> combine heads -> A @ Wo.T + bo

Sharding: 8 cores = 4 batches x 2 query-halves. Softmax over heads is local
to each (n,m) score position, so with all heads on one core there is no
cross-core coupling -> no collectives. K/V work for a batch is duplicated
across its 2 cores.

Per-core pipeline (NQ=1024 query rows, NK=2048 key rows, D=512, H=8, dk=64):
  prologue: K path only (PE-transpose bf16 tiles + projection with host
           pre-transposed, 1/sqrt(dk)-pre-scaled Wq.T so exp runs with
           scale=1), then Q transpose + the nch=0 half of the q
           projection. V and the q nch=1 half stream inside the loop.
  loop over 32 (n-chunk 512, m-tile 128) steps, software-pipelined
  (norm at lag 3, A^T at lag 5):
    - 4 row-packed score-matmul pairs + ACT exps (the relay heartbeat),
      with the lag-5 A^T half-batches interleaved into the relay's PE
      idle gaps; one streamed V-tile (load/cast/PE-transpose/project)
      per step for the first 16 steps; the q nch=1 projection spread
      one e-tile per step over steps 8-11; one spread-out output-
      projection slice per step after a sweep finishes.
    - norm(c-3): cross-head sum via one SBUF->SBUF DMA copy + one DMA
      accumulate (gpsimd SWDGE; the adds run on DMA engines, and E's
      last reader is the accum so the E ring never stalls the exps),
      two DVE fold adds + one GpSimd f32 add, reciprocal_approx_fast,
      bf16 cast, and the broadcast normalize mul in two half tiles so
      A can start after the first half. No PE reduction work.
  drain: remaining norms use PE identity-matmul head-sums (the PE is
  idle there; avoids the DMA-chain latency), then A^T + output proj.
"""

import sys

sys.path.insert(0, "/opt/trn_rl_repo")

import math
from contextlib import ExitStack

import numpy as np

import concourse.bass as bass
from concourse.bacc import Bacc
import concourse.mybir as mybir
import concourse.tile as tile
from concourse.masks import make_identity

F32 = mybir.dt.float32
F32R = mybir.dt.float32r
BF16 = mybir.dt.bfloat16
ADD = mybir.AluOpType.add
MULT = mybir.AluOpType.mult

B, N, D, H = 4, 2048, 512, 8
DK = D // H           # 64
NQ = N // 2           # 1024 query rows per core
NK = N                # 2048 key rows per core
NCH = 512             # n-chunk (score matmul free dim)
N_CHUNKS = NQ // NCH  # 2
MT = NK // 128        # 16 m-tiles
ET = D // 128         # 4 e-tiles (= head pairs)
SCALE = 1.0 / math.sqrt(DK)


def r32(ap):
    return ap.bitcast(F32R)


def build_nc(repeat: int | None = None) -> bass.Bass:
    nc = Bacc()

    Qd = nc.dram_tensor("q_in", [NQ, D], F32, kind="ExternalInput")
    Kd = nc.dram_tensor("k_in", [NK, D], F32, kind="ExternalInput")
    Vd = nc.dram_tensor("v_in", [NK, D], F32, kind="ExternalInput")
    WqTd = nc.dram_tensor("wqt", [D, D], F32, kind="ExternalInput")  # Wq.T [d, e]
    WqTsd = nc.dram_tensor("wqts", [D, D], F32, kind="ExternalInput")  # Wq.T/sqrt(dk)
    WoTd = nc.dram_tensor("wot", [D, D], F32, kind="ExternalInput")  # Wo.T [e, eo]
    bqd = nc.dram_tensor("bq", [1, D], F32, kind="ExternalInput")
    bqsd = nc.dram_tensor("bqs", [1, D], F32, kind="ExternalInput")  # bq/sqrt(dk)
    bod = nc.dram_tensor("bo", [1, D], F32, kind="ExternalInput")
    OUT = nc.dram_tensor("out", [NQ, D], F32, kind="ExternalOutput")

    with ExitStack() as ctx:
        tc = ctx.enter_context(tile.TileContext(nc))
        _emit(ctx, tc, Qd, Kd, Vd, WqTd, WqTsd, WoTd, bqd, bqsd, bod, OUT,
              repeat=repeat)

    nc.finalize()
    return nc


def _emit(ctx, tc, Qd, Kd, Vd, WqTd, WqTsd, WoTd, bqd, bqsd, bod, OUT,
          repeat=None):
    nc = tc.nc

    # ---------------------------------------------------------- constants
    const_pool = ctx.enter_context(tc.tile_pool(name="const", bufs=1))

    ident = const_pool.tile([128, 128], F32, name="ident")
    make_identity(nc, ident)
    ident_bf = const_pool.tile([128, 128], BF16, name="ident_bf")
    make_identity(nc, ident_bf)

    # bq with e on partitions: element (p, t) = bq[t*128 + p]
    bq_cols = const_pool.tile([128, ET], F32, name="bq_cols")
    nc.sync.dma_start(bq_cols[:, :], bqd[0, :].rearrange("(t p) -> p t", p=128))
    bqs_cols = const_pool.tile([128, ET], F32, name="bqs_cols")
    nc.sync.dma_start(bqs_cols[:, :], bqsd[0, :].rearrange("(t p) -> p t", p=128))

    bq_bcast = const_pool.tile([128, D], F32, name="bq_bcast")
    bo_bcast = const_pool.tile([128, D], F32, name="bo_bcast")

    wqt_bf = []   # Wq.T bf16 tiles, d on partitions (q/v path)
    wqts_bf = []  # Wq.T/sqrt(dk) bf16 tiles (k path)
    wot_bf = []   # Wo.T bf16 tiles, e on partitions
    for t in range(ET):
        wqt_bf.append(
            const_pool.tile([128, D], BF16, name=f"wqtb{t}", tag=f"wqtb{t}")
        )
        wqts_bf.append(
            const_pool.tile([128, D], BF16, name=f"wqtsb{t}", tag=f"wqtsb{t}")
        )
        wot_bf.append(
            const_pool.tile([128, D], BF16, name=f"wotb{t}", tag=f"wotb{t}")
        )

    with tc.tile_pool(name="setup_stage", bufs=2) as sstage:
        for bias_d, dst in ((bqd, bq_bcast), (bod, bo_bcast)):
            nc.sync.dma_start(dst[:, :], bias_d[0, :].partition_broadcast(128))
        for src_d, dsts in ((WqTd, wqt_bf), (WqTsd, wqts_bf), (WoTd, wot_bf)):
            for t in range(ET):
                wstage = sstage.tile([128, D], F32, name="wstage", tag="wstage")
                nc.sync.dma_start(wstage[:, :], src_d[t * 128 : (t + 1) * 128, :])
                nc.vector.tensor_copy(dsts[t][:, :], wstage[:, :])

    # --------------------------------------------------- persistent SBUF
    qp_pool = ctx.enter_context(tc.tile_pool(name="qp", bufs=ET))
    kp_pool = ctx.enter_context(tc.tile_pool(name="kp", bufs=ET))
    vp_pool = ctx.enter_context(tc.tile_pool(name="vp", bufs=MT))
    qpT = [qp_pool.tile([128, NQ], BF16, name=f"qpT{t}", tag="qpT") for t in range(ET)]
    kpT = [kp_pool.tile([128, NK], BF16, name=f"kpT{t}", tag="kpT") for t in range(ET)]
    vp = [vp_pool.tile([128, D], BF16, name=f"vp{m}", tag="vp") for m in range(MT)]

    # ----------------------------------------------------------- phase 1
    def load_transpose(stage_pool, ps_pool, Xd, xT_all, n_rows):
        """DMA [n_rows, D] fp32 from DRAM, PE-transpose (bf16) into a single
        [128, ET*n_rows] tensor (d-tile-major along free); one scatter-copy
        evacuation per 128-row block."""
        xT3 = xT_all[:, :].rearrange("p (t n) -> p t n", t=ET)
        for ntile in range(n_rows // 128):
            st = stage_pool.tile([128, D], F32, name="x_stage", tag="stage")
            dma_eng = nc.sync if ntile % 2 == 0 else nc.scalar
            dma_eng.dma_start(st[:, :], Xd[ntile * 128 : (ntile + 1) * 128, :])
            st_bf = stage_pool.tile([128, D], BF16, name="x_stage_bf", tag="stage_bf")
            nc.scalar.copy(st_bf[:, :], st[:, :])
            ps = ps_pool.tile([128, D], BF16, name="ps_tr", tag="ps_s")
            for dt_ in range(ET):
                nc.tensor.transpose(
                    ps[:, dt_ * 128 : (dt_ + 1) * 128],
                    st_bf[:, dt_ * 128 : (dt_ + 1) * 128],
                    ident_bf[:, :],
                )
            nc.vector.tensor_copy(
                xT3[:, :, ntile * 128 : (ntile + 1) * 128],
                ps[:, :].rearrange("p (t n) -> p t n", t=ET),
            )

    def project_T(ps_pool, xT_all, xpT, n_rows, w_bf, b_cols, nchs=None,
                  ps_tag="psA", ets=None):
        """xpT[et][e, n] = sum_d W[d, e] xT[d, n] + b[e]  (bf16)."""
        if nchs is None:
            nchs = range(n_rows // NCH)
        if ets is None:
            ets = range(ET)
        for et in ets:
            for nch in nchs:
                ps = ps_pool.tile([128, NCH], F32, name="ps_proj", tag=ps_tag)
                for dt_ in range(ET):
                    base = dt_ * n_rows + nch * NCH
                    nc.tensor.matmul(
                        ps[:, :],
                        w_bf[dt_][:, et * 128 : (et + 1) * 128],
                        xT_all[:, base : base + NCH],
                        start=(dt_ == 0),
                        stop=(dt_ == ET - 1),
                    )
                nc.vector.tensor_scalar_add(
                    xpT[et][:, nch * NCH : (nch + 1) * NCH],
                    ps[:, :],
                    b_cols[:, et : et + 1],
                )

    stage_pool = ctx.enter_context(tc.tile_pool(name="stage", bufs=4))
    xtq_pool = ctx.enter_context(tc.tile_pool(name="xtq", bufs=1))
    xtk_pool = ctx.enter_context(tc.tile_pool(name="xtk", bufs=1))
    e_pool = ctx.enter_context(tc.tile_pool(name="ework", bufs=5))
    p_pool = ctx.enter_context(tc.tile_pool(name="pwork", bufs=5))
    r_pool = ctx.enter_context(tc.tile_pool(name="rwork", bufs=2))
    a_pool = ctx.enter_context(tc.tile_pool(name="abuf", bufs=ET))
    o_pool = ctx.enter_context(tc.tile_pool(name="ostage", bufs=2))
    # PSUM: 8 banks total. ps_s pool: 2 slots x [128,1024]f32 (2 banks each);
    # phase-1 transposes + out-proj share the slots via the same tag.
    # ps_a pool: 4 slots x 1 bank; phase-1 projections share via tag.
    ps_s_pool = ctx.enter_context(tc.tile_pool(name="ps_s", bufs=2, space="PSUM"))
    ps_a_pool = ctx.enter_context(tc.tile_pool(name="ps_a", bufs=ET, space="PSUM"))

    ps_t_pool = ps_s_pool
    ps_p_pool = ps_a_pool

    def body():
        # K first: scores need the full kpT before step 0
        kT = xtk_pool.tile([128, ET * NK], BF16, name="kT", tag="kT")
        load_transpose(stage_pool, ps_t_pool, Kd, kT, NK)
        project_T(ps_p_pool, kT, kpT, NK, wqts_bf, bqs_cols)

        # Q: transpose all tiles, but project only the nch=0 half now;
        # the nch=1 half is projected mid-loop before it is needed
        qT = xtq_pool.tile([128, ET * NQ], BF16, name="qT", tag="qT")
        load_transpose(stage_pool, ps_t_pool, Qd, qT, NQ)
        project_T(ps_p_pool, qT, qpT, NQ, wqt_bf, bq_cols, nchs=(0,))

        def emit_v_tile(m):
            """Stream one V m-tile: load, cast (DVE), PE-transpose,
            project into vp[m]. Runs inside the phase-2 loop; vp[m] is
            first consumed by A at step m + A_LAG."""
            st = stage_pool.tile([128, D], F32, name="x_stage", tag="stage")
            dma_eng = nc.sync if m % 2 == 0 else nc.scalar
            dma_eng.dma_start(st[:, :], Vd[m * 128 : (m + 1) * 128, :])
            st_bf = stage_pool.tile([128, D], BF16, name="x_stage_bf", tag="stage_bf")
            nc.vector.tensor_copy(st_bf[:, :], st[:, :])
            tr = stage_pool.tile([128, D], BF16, name="v_tr", tag="v_tr")
            ps_tr = ps_t_pool.tile([128, D], BF16, name="ps_tr", tag="ps_s")
            for dt_ in range(ET):
                nc.tensor.transpose(
                    ps_tr[:, dt_ * 128 : (dt_ + 1) * 128],
                    st_bf[:, dt_ * 128 : (dt_ + 1) * 128],
                    ident_bf[:, :],
                )
            nc.vector.tensor_copy(tr[:, :], ps_tr[:, :])
            ps = ps_t_pool.tile([128, D], F32, name="ps_vp", tag="ps_s")
            for dt_ in range(ET):
                nc.tensor.matmul(
                    ps[:, :],
                    tr[:, dt_ * 128 : (dt_ + 1) * 128],
                    wqt_bf[dt_][:, :],
                    start=(dt_ == 0),
                    stop=(dt_ == ET - 1),
                )
            nc.vector.tensor_tensor(vp[m][:, :], ps[:, :], bq_bcast[:, :], ADD)

        # ------------------------------------------------------- phase 2
        def emit_A_half(psA, mt, Phalves, j):
            """A^T accumulation for head pairs (2j, 2j+1); col-packed.
            Emitted interleaved into the score relay's PE idle gaps."""
            for pair in (2 * j, 2 * j + 1):
                Ph = Phalves[pair // 2]
                hbase = (pair % 2) * 2
                for half in range(2):
                    h = 2 * pair + half
                    nc.tensor.matmul(
                        psA[pair][64 * half : 64 * (half + 1), :],
                        vp[mt][:, h * DK : (h + 1) * DK],
                        Ph[:, (hbase + half) * NCH : (hbase + half + 1) * NCH],
                        start=(mt == 0),
                        stop=(mt == MT - 1),
                        tile_position=(0, 64 * half),
                        # the sim's zero-region tracker can't see the
                        # partition offset; the two col-packed halves of
                        # one bank are distinct accumulation groups
                        skip_group_check=True,
                    )

        def emit_score_pair(E, nch, mt, pair):
            """Row-packed score matmuls + exp for one head pair."""
            nsl = slice(nch * NCH, (nch + 1) * NCH)
            msl = slice(mt * 128, (mt + 1) * 128)
            ps_s = ps_s_pool.tile([128, 2 * NCH], F32, name="ps_s", tag="ps_s")
            for half in range(2):
                hsl = slice(64 * half, 64 * (half + 1))
                nc.tensor.matmul(
                    ps_s[:, half * NCH : (half + 1) * NCH],
                    kpT[pair][hsl, msl],
                    qpT[pair][hsl, nsl],
                    tile_position=(64 * half, 0),
                )
            nc.scalar.activation(
                E[:, pair * 2 * NCH : (pair + 1) * 2 * NCH],
                ps_s[:, :],
                mybir.ActivationFunctionType.Exp,
            )

        def emit_norm(E):
            """Cross-head sum: two parallel SBUF->SBUF DMA copies pull the
            E halves out (so the E buffer frees right away and the exp
            relay never stalls on it), one DMA accumulate folds them, then
            DVE + GpSimd merge adds, reciprocal + cast + the broadcast
            normalize mul (DVE). No PE work. Returns P halves."""
            T1 = r_pool.tile([128, 4 * NCH], BF16, name="Tsum1", tag="Tsum1")
            nc.gpsimd.dma_start(T1[:, :], E[:, 0 : 4 * NCH])
            # T1 = (b0+b4 | b1+b5 | b2+b6 | b3+b7); E's last reader is
            # this accum's source -- E frees after just two chain links
            nc.gpsimd.dma_start(
                T1[:, :], E[:, 4 * NCH : 8 * NCH], accum_op=ADD
            )
            # fold the four NCH blocks: u = B0+B1, u2 = B2+B3 (DVE, bf16)
            u = r_pool.tile([128, NCH], BF16, name="u_sum", tag="u_sum")
            nc.vector.tensor_tensor(
                u[:, :], T1[:, 0:NCH], T1[:, NCH : 2 * NCH], ADD
            )
            u2 = r_pool.tile([128, NCH], BF16, name="u2_sum", tag="u2_sum")
            nc.vector.tensor_tensor(
                u2[:, :], T1[:, 2 * NCH : 3 * NCH], T1[:, 3 * NCH : 4 * NCH], ADD
            )
            # S = u + u2  (f32, on GpSimd -- SBUF only)
            s_f = r_pool.tile([128, NCH], F32, name="s_f", tag="s_f")
            nc.gpsimd.tensor_tensor(s_f[:, :], u[:, :], u2[:, :], ADD)
            r_f = r_pool.tile([128, NCH], F32, name="r_f", tag="r_f")
            nc.vector.reciprocal_approx_fast(r_f[:, :], s_f[:, :])
            r_bf = r_pool.tile([128, NCH], BF16, name="r_bf", tag="r_bf")
            nc.vector.tensor_copy(r_bf[:, :], r_f[:, :])
            # normalize in two half tiles so downstream A matmuls can
            # start after the first half
            HH = H // 2
            Phalves = []
            for j in range(2):
                Ph = p_pool.tile([128, HH * NCH], BF16, name=f"P{j}", tag=f"P{j}")
                nc.vector.tensor_tensor(
                    Ph[:, :].rearrange("p (h n) -> p h n", h=HH),
                    E[:, j * HH * NCH : (j + 1) * HH * NCH].rearrange(
                        "p (h n) -> p h n", h=HH
                    ),
                    r_bf[:, None, :].broadcast_to([128, HH, NCH]),
                    MULT,
                )
                Phalves.append(Ph)
            return Phalves

        def emit_norm_pe(E):
            """Drain-time norm: head-sum via PE identity-matmul
            accumulation (the PE is idle during the drain; avoids the
            DMA-chain latency), then the usual recip/cast/mul chain."""
            ps_sum = ps_s_pool.tile([128, NCH], F32, name="ps_sum", tag="ps_s")
            for h in range(H):
                nc.tensor.matmul(
                    ps_sum[:, :],
                    ident_bf[:, :],
                    E[:, h * NCH : (h + 1) * NCH],
                    start=(h == 0),
                    stop=(h == H - 1),
                )
            r_f = r_pool.tile([128, NCH], F32, name="r_f", tag="r_f")
            nc.vector.reciprocal_approx_fast(r_f[:, :], ps_sum[:, :])
            r_bf = r_pool.tile([128, NCH], BF16, name="r_bf", tag="r_bf")
            nc.vector.tensor_copy(r_bf[:, :], r_f[:, :])
            HH = H // 2
            Phalves = []
            for j in range(2):
                Ph = p_pool.tile([128, HH * NCH], BF16, name=f"P{j}", tag=f"P{j}")
                nc.vector.tensor_tensor(
                    Ph[:, :].rearrange("p (h n) -> p h n", h=HH),
                    E[:, j * HH * NCH : (j + 1) * HH * NCH].rearrange(
                        "p (h n) -> p h n", h=HH
                    ),
                    r_bf[:, None, :].broadcast_to([128, HH, NCH]),
                    MULT,
                )
                Phalves.append(Ph)
            return Phalves

        def emit_evac(psA):
            # evacuate A^T: psA[pair] partitions = e-rows 128*pair..+127
            a_bf = [
                a_pool.tile([128, NCH], BF16, name=f"a_bf{p}", tag="a_bf")
                for p in range(ET)
            ]
            # GpSimd cannot read PSUM; evacuation stays on DVE
            for p in range(ET):
                nc.vector.tensor_copy(a_bf[p][:, :], psA[p][:, :])
            return a_bf

        def emit_out(nch, a_bf, nt2):
            # output projection: out[n, eo] = sum_e A^T[e, n] WoT[e, eo] + bo
            ps_o = ps_s_pool.tile([128, D], F32, name="ps_o", tag="ps_s")
            for p in range(ET):
                nc.tensor.matmul(
                    ps_o[:, :],
                    a_bf[p][:, nt2 * 128 : (nt2 + 1) * 128],
                    wot_bf[p][:, :],
                    start=(p == 0),
                    stop=(p == ET - 1),
                )
            o_st = o_pool.tile([128, D], F32, name="o_st", tag="o_st")
            nc.vector.tensor_tensor(o_st[:, :], ps_o[:, :], bo_bcast[:, :], ADD)
            nc.sync.dma_start(
                OUT[nch * NCH + nt2 * 128 : nch * NCH + (nt2 + 1) * 128, :],
                o_st[:, :],
            )

        # deep software pipeline over all (n-chunk, m-tile) steps:
        #   step c: scores+exp(c) | norm(c-2) | A(c-4)
        # The DMA-accum head-sum chain has ~7us latency and the full norm
        # chain ~12us; lag 2 for norm and lag 4 for A absorb it. A(c-4) is
        # emitted after the scores so the PE queue never head-of-line
        # blocks the score->exp relay.
        NORM_LAG = 3
        A_LAG = 5
        steps = [(nch, mt) for nch in range(N_CHUNKS) for mt in range(MT)]
        psA_of = {}
        pend_norm = []  # [(nch, mt, E), ...]  emit_norm at lag NORM_LAG
        pend_A = []     # [(nch, mt, P), ...]  emit_A at lag A_LAG

        pend_fin = []  # [(nch, a_bf, nt2), ...] outproj spread 1/step

        def emit_step(ci, nch, mt, arec):
            """One pipeline step: the 4 score pairs + exps of (nch, mt)
            with the A(c-A_LAG) half-batches interleaved into the score
            relay's PE idle gaps, then the streamed V tile / spread-out
            deferred q projection / one spread-out output-projection
            slice, then the norm(c-NORM_LAG) chain."""
            E = e_pool.tile([128, H * NCH], BF16, name="E", tag="E")
            emit_score_pair(E, nch, mt, 0)
            emit_score_pair(E, nch, mt, 1)
            if arec is not None:
                a_nch, a_mt, a_P = arec
                emit_A_half(psA_of[a_nch], a_mt, a_P, 0)
            emit_score_pair(E, nch, mt, 2)
            if arec is not None:
                emit_A_half(psA_of[a_nch], a_mt, a_P, 1)
                if a_mt == MT - 1:
                    a_bf = emit_evac(psA_of[a_nch])
                    pend_fin.extend(
                        (a_nch, a_bf, nt2) for nt2 in range(NCH // 128)
                    )
            emit_score_pair(E, nch, mt, 3)
            if ci is not None and ci < MT:
                emit_v_tile(ci)  # vp[ci] first consumed at step ci+A_LAG
            if ci is not None and 8 <= ci < 8 + ET:
                # project one e-tile of the q second half per step
                # (spread to avoid a PE spike). Must NOT use the psA tag
                # (held by live A accumulators).
                project_T(ps_s_pool, qT, qpT, NQ, wqt_bf, bq_cols, nchs=(1,),
                          ps_tag="ps_s", ets=(ci - 8,))
            if pend_fin:
                emit_out(*pend_fin.pop(0))
            return E

        for ci, (nch, mt) in enumerate(steps):
            if mt == 0:
                psA_of[nch] = [
                    ps_a_pool.tile([128, NCH], F32, name=f"psA{p}", tag="psA")
                    for p in range(ET)
                ]
            arec = pend_A.pop(0) if len(pend_A) > A_LAG - NORM_LAG - 1 else None
            E = emit_step(ci, nch, mt, arec)
            pend_norm.append((nch, mt, E))
            if len(pend_norm) > NORM_LAG:
                n_nch, n_mt, n_E = pend_norm.pop(0)
                pend_A.append((n_nch, n_mt, emit_norm(n_E)))
        # drain: the PE is idle here, so the remaining head-sums run as
        # identity-matmul accumulations (no DMA-chain latency)
        while pend_norm:
            n_nch, n_mt, n_E = pend_norm.pop(0)
            pend_A.append((n_nch, n_mt, emit_norm_pe(n_E)))
        for a_nch, a_mt, a_P in pend_A:
            emit_A_half(psA_of[a_nch], a_mt, a_P, 0)
            emit_A_half(psA_of[a_nch], a_mt, a_P, 1)
            if a_mt == MT - 1:
                a_bf = emit_evac(psA_of[a_nch])
                pend_fin.extend((a_nch, a_bf, nt2) for nt2 in range(NCH // 128))
        for rec in pend_fin:
            emit_out(*rec)

    if repeat:
        # timing variant: loop the whole kernel on-device so execution time
        # dominates the (noisy, ~1ms) per-dispatch tunnel overhead
        with tc.For_i(0, repeat, 1):
            body()
    else:
        body()


# ---------------------------------------------------------------------------
# host wrapper

_CACHED = {}


def _get_nc():
    if "nc" not in _CACHED:
        _CACHED["nc"] = build_nc()
    return _CACHED["nc"]


def make_in_maps(Q, K, V, Wq, bq, Wo, bo):
    Q = np.asarray(Q, dtype=np.float32)
    K = np.asarray(K, dtype=np.float32)
    V = np.asarray(V, dtype=np.float32)
    WqT = np.ascontiguousarray(np.asarray(Wq, np.float32).T)
    WqTs = np.ascontiguousarray(WqT * np.float32(SCALE))
    WoT = np.ascontiguousarray(np.asarray(Wo, np.float32).T)
    bq = np.ascontiguousarray(np.asarray(bq, np.float32)).reshape(1, D)
    bqs = np.ascontiguousarray(bq * np.float32(SCALE))
    bo = np.ascontiguousarray(np.asarray(bo, np.float32)).reshape(1, D)

    in_maps = []
    for c in range(8):
        b, half = divmod(c, 2)
        in_maps.append(
            {
                "q_in": np.ascontiguousarray(Q[b, half * NQ : (half + 1) * NQ]),
                "k_in": np.ascontiguousarray(K[b]),
                "v_in": np.ascontiguousarray(V[b]),
                "wqt": WqT,
                "wqts": WqTs,
                "wot": WoT,
                "bq": bq,
                "bqs": bqs,
                "bo": bo,
            }
        )
    return in_maps


def kernel(Q, K, V, Wq, bq, Wo, bo):
    from concourse import bass_utils

    nc = _get_nc()
    in_maps = make_in_maps(Q, K, V, Wq, bq, Wo, bo)
    res = bass_utils.run_bass_kernel_spmd(nc, in_maps, core_ids=list(range(8)))

    out = np.empty((B, N, D), np.float32)
    for c in range(8):
        b, half = divmod(c, 2)
        out[b, half * NQ : (half + 1) * NQ] = res.results[c]["out"]
    return out

